# revision 5
# baseline (speedup 1.0000x reference)
"""Self-attention block (B=16, S=1024, C=512, H=8, D=64) on 8 NeuronCores.

Data-parallel over batch: core i handles batches [2i, 2i+1]. No collectives.

Per-core pipeline (all on-chip after the initial DMAs):
  qkv proj -> q,k feature-major [d, s], v token-major [s, d] padded to 128
  cols per head (64 v + ones col + pad -> FWL-eligible stationary); scores
  computed transposed S'[j, i] = k . q so exp(S') feeds the P@V matmul
  directly as lhsT; softmax skips max-subtraction (logits bounded ~+-4);
  deferred per-head normalization divides O^T rows by the row-sums
  (respread over 128 lanes via a DRAM bounce for the DVE reciprocal,
  broadcast back in bf16, in-place multiply); output projection consumes
  the normalized heads straight out of SBUF. The value-path bias is folded
  through attention into the output bias (exact: softmax rows sum to 1).

Scheduling (v2): fully software-pipelined. Each head's scores run TWO jc
chunks ahead of its P@V, and the next head's first two scores chunks are
emitted immediately after the previous head's last P@V, so the ACT (exp)
engine always has a backlog and the PE never waits an exp latency at head
boundaries. All projection work (qkv chunks of the next batch, v chunks,
the previous batch's output projection) is spread as small filler units
between pipeline slots instead of big blocks, which keeps the ACT engine
fed across batch boundaries. The final batch's output projection runs
two-phase (cc=0..2 staged into SBUF during head 7, cc=3 added after) so
the last normalize's DRAM bounce hides under real PE work. Row-sums are
DMA'd straight out of PSUM (no DVE staging copy).

Dtypes: bf16 matmuls throughout. NOTE: the chip enforces a package power
cap -- schedules that pack the PE much past ~80% active trip a 50%
utilization clamp and run slower.
"""

import numpy as np

import concourse.bacc as bacc
import concourse.tile as tile
import concourse.mybir as mybir
from concourse.bass_utils import run_bass_kernel_spmd

B, S, C, H, D = 16, 1024, 512, 8, 64
NCORES = 8
BPC = B // NCORES  # batches per core
F32 = mybir.dt.float32
ADT = mybir.dt.bfloat16

SCJ = 8  # S/128 chunks (token/key chunks)
CCH = 4  # C/128 chunks (model-dim chunks)
FCH = 8  # (2C)/128 chunks of q|k features
VW = H * 128  # 1024: v row width per jc; per head 128 cols = 64 d + ones + pad


def _register_ntff_hook():
    import sys, types

    if "antenv.axon_hooks" in sys.modules:
        return
    try:
        import trn_agent_boot.trn_boot as tb

        hook = [None]
        mod = types.ModuleType("antenv.axon_hooks")
        mod.set_axon_ntff_profile_hook = lambda h: hook.__setitem__(0, h)
        mod.get_axon_ntff_profile_hook = lambda: hook[0]
        sys.modules["antenv.axon_hooks"] = mod
        mod.set_axon_ntff_profile_hook(
            tb._ntff_profile_via_ctypes("/opt/axon/libaxon_pjrt.so")
        )
    except Exception:
        pass


def build():
    nc = bacc.Bacc("TRN2", target_bir_lowering=False, debug=False)

    xT = nc.declare_dram_parameter("xT", [BPC, C, S], ADT, isOutput=False)
    wqkvT = nc.declare_dram_parameter("wqkvT", [C, 3 * C], ADT, isOutput=False)
    wouT = nc.declare_dram_parameter("wouT", [C, C], ADT, isOutput=False)
    bqk = nc.declare_dram_parameter("bqk", [128, FCH], F32, isOutput=False)
    beff = nc.declare_dram_parameter("beff", [C], F32, isOutput=False)
    y = nc.declare_dram_parameter("y", [BPC, S, C], F32, isOutput=True)

    from contextlib import ExitStack

    with tile.TileContext(nc) as tc, ExitStack() as ctx:
        ctx.enter_context(
            nc.allow_low_precision(reason="bf16 matmul operand staging")
        )
        consts = ctx.enter_context(tc.tile_pool(name="consts", bufs=1))
        xpool = ctx.enter_context(tc.tile_pool(name="x", bufs=2))
        qkpool = ctx.enter_context(tc.tile_pool(name="qkt", bufs=17))
        vpool = ctx.enter_context(tc.tile_pool(name="v", bufs=2))
        ppool = ctx.enter_context(tc.tile_pool(name="p", bufs=4))
        opool = ctx.enter_context(tc.tile_pool(name="o", bufs=2))
        rpool = ctx.enter_context(tc.tile_pool(name="r", bufs=4))
        spool = ctx.enter_context(tc.tile_pool(name="s", bufs=2))
        ypool = ctx.enter_context(tc.tile_pool(name="y", bufs=11))
        bcpool = ctx.enter_context(tc.tile_pool(name="bc", bufs=3))
        drpool = ctx.enter_context(tc.tile_pool(name="dr", bufs=4, space="DRAM"))
        ps_a = ctx.enter_context(tc.tile_pool(name="ps_a", bufs=3, space="PSUM"))
        ps_o = ctx.enter_context(tc.tile_pool(name="ps_o", bufs=1, space="PSUM"))

        # --- boot DMAs: interleave x chunks (sync queue) with q|k thirds of
        # w_qkv (gpsimd queue) so the first scores' deps land earliest; the
        # v third, w_out, and biases follow.
        wq_sb = consts.tile([128, CCH * 3 * C], ADT)  # [c%128, cc*1536 + f]
        bqk_sb = consts.tile([128, FCH], F32)
        nc.sync.dma_start(out=bqk_sb, in_=bqk[:, :])
        x_tiles = [None, None]
        x_tiles[0] = xpool.tile([128, CCH * S], ADT, tag="x", name="x0")
        for cc in range(CCH):
            nc.sync.dma_start(
                out=x_tiles[0][:, cc * S : (cc + 1) * S],
                in_=xT[0][cc * 128 : (cc + 1) * 128, :],
            )
            nc.gpsimd.dma_start(
                out=wq_sb[:, cc * 1536 : cc * 1536 + 512],
                in_=wqkvT[cc * 128 : (cc + 1) * 128, 0:512],
            )
            nc.gpsimd.dma_start(
                out=wq_sb[:, cc * 1536 + 512 : cc * 1536 + 1024],
                in_=wqkvT[cc * 128 : (cc + 1) * 128, 512:1024],
            )
        for cc in range(CCH):  # v third, needed a few us later
            nc.gpsimd.dma_start(
                out=wq_sb[:, cc * 1536 + 1024 : cc * 1536 + 1536],
                in_=wqkvT[cc * 128 : (cc + 1) * 128, 1024:1536],
            )
        beff_sb = consts.tile([128, C], F32)
        nc.gpsimd.dma_start(out=beff_sb, in_=beff[:].partition_broadcast(128))
        wo_sb = consts.tile([128, CCH * C], ADT)  # [c%128, cc*512 + f]
        nc.sync.dma_start(
            out=wo_sb.rearrange("p (cc f) -> p cc f", cc=CCH),
            in_=wouT[:, :].rearrange("(cc p) f -> p cc f", p=128),
        )

        def emit_x(b):
            # x^T for batch b: [c, s] as [c%128, cc*1024 + s]
            x_sb = xpool.tile([128, CCH * S], ADT, tag="x", name=f"x{b}")
            for cc in range(CCH):
                nc.sync.dma_start(
                    out=x_sb[:, cc * S : (cc + 1) * S],
                    in_=xT[b][cc * 128 : (cc + 1) * 128, :],
                )
            x_tiles[b] = x_sb

        qk_tiles = {0: [None] * FCH, 1: [None] * FCH}

        def emit_qk_chunk(b, fc):
            # q/k projection chunk: qkT[fc] = W_qk^T[:,fc].T @ x^T + b
            x_sb = x_tiles[b]
            qt = qkpool.tile([128, S], ADT, tag="qkt", name=f"qkt{b}_{fc}")
            ps = ps_a.tile([128, 1024], F32, tag="ps_a", name=f"psq{b}_{fc}")
            for ih in range(2):
                for cc in range(CCH):
                    nc.tensor.matmul(
                        ps[:, ih * 512 : (ih + 1) * 512],
                        lhsT=wq_sb[:, cc * 1536 + fc * 128 : cc * 1536 + (fc + 1) * 128],
                        rhs=x_sb[:, cc * S + ih * 512 : cc * S + ih * 512 + 512],
                        start=(cc == 0),
                        stop=(cc == CCH - 1),
                    )
            nc.vector.tensor_scalar_add(
                out=qt, in0=ps[:, :], scalar1=bqk_sb[:, fc : fc + 1]
            )
            qk_tiles[b][fc] = qt

        v_tiles = [None, None]

        def emit_v_alloc(b):
            # v token-major [s%128, jc*1024 + h*128 + d]; cols 64..127 of each
            # head block = ones (col 64 is the row-sums column; 65..127 pad the
            # stationary to 128 for fast weight load, their psum rows unread).
            v_sb = vpool.tile([128, SCJ * VW], ADT, tag="v", name=f"v{b}")
            v_view = v_sb.rearrange("p (jc h dd) -> p jc h dd", jc=SCJ, h=H)
            eng = nc.vector if b == 0 else nc.gpsimd
            eng.memset(v_view[:, :, :, D : 128], 1.0)
            v_tiles[b] = v_sb

        def emit_v_chunk(b, jc):
            x_sb = x_tiles[b]
            v_view = v_tiles[b].rearrange("p (jc h dd) -> p jc h dd", jc=SCJ, h=H)
            ps = ps_a.tile([128, 1024], F32, tag="ps_a", name=f"psv{b}_{jc}")
            for cc in range(CCH):
                nc.tensor.matmul(
                    ps[:, 0:512],
                    lhsT=x_sb[:, cc * S + jc * 128 : cc * S + (jc + 1) * 128],
                    rhs=wq_sb[:, cc * 1536 + 1024 : cc * 1536 + 1536],
                    start=(cc == 0),
                    stop=(cc == CCH - 1),
                )
            # DVE, not gpsimd: the Q7 has no PSUM port
            nc.vector.tensor_copy(
                out=v_view[:, jc, :, 0:D],
                in_=ps[:, 0:512].rearrange("p (h d) -> p h d", h=H),
            )

        pts = {}
        pos = {}
        o_sbs = {}
        sums_drs = {}

        def emit_s(b, h, jc):
            # scores S'[j, i] = k . q (transposed), then P' = exp(scale * S')
            fq = h // 2
            fk = 4 + h // 2
            pb = (h % 2) * 64
            ps = ps_a.tile([128, 1024], F32, tag="ps_a", name=f"pss{b}_{h}_{jc}")
            for ih in range(2):
                nc.tensor.matmul(
                    ps[:, ih * 512 : (ih + 1) * 512],
                    lhsT=qk_tiles[b][fk][pb : pb + 64, jc * 128 : (jc + 1) * 128],
                    rhs=qk_tiles[b][fq][pb : pb + 64, ih * 512 : ih * 512 + 512],
                    start=True,
                    stop=True,
                )
            pt = ppool.tile([128, 1024], ADT, tag="p", name=f"pt{b}_{h}_{jc}")
            nc.scalar.activation(
                out=pt, in_=ps[:, :],
                func=mybir.ActivationFunctionType.Exp,
                scale=float(D) ** -0.5,
            )
            pts[(b, h, jc)] = pt

        def emit_p(b, h, jc):
            # O^T[d, i] += V_ext^T @ P'  (row 64 = row-sums; rows 65+ unread)
            if jc == 0:
                pos[(b, h)] = ps_o.tile([128, 1024], F32, tag="ps_o", name=f"po{b}_{h}")
            po = pos[(b, h)]
            pt = pts.pop((b, h, jc))
            v_sb = v_tiles[b]
            for ih in range(2):
                nc.tensor.matmul(
                    po[:, ih * 512 : (ih + 1) * 512],
                    lhsT=v_sb[:, jc * VW + h * 128 : jc * VW + (h + 1) * 128],
                    rhs=pt[:, ih * 512 : (ih + 1) * 512],
                    start=(jc == 0),
                    stop=(jc == SCJ - 1),
                )

        def emit_evac(b, h):
            # row-sums first (they gate the normalize chain; DMA cannot read
            # PSUM so stage through SBUF), then the unnormalized O^T rows
            po = pos.pop((b, h))
            hh = h % 2
            sums_sb = spool.tile([1, S], F32, tag="sums", name=f"sm{b}_{h}")
            nc.vector.tensor_copy(out=sums_sb, in_=po[64:65, :])
            sums_dr = drpool.tile([S], F32, tag="sdr", name=f"sdr{b}_{h}")
            nc.sync.dma_start(out=sums_dr[:].unsqueeze(0), in_=sums_sb)
            sums_drs[(b, h)] = sums_dr
            o_sb = o_sbs[b]
            nc.vector.tensor_copy(
                out=o_sb[hh * 64 : (hh + 1) * 64, (h // 2) * S : (h // 2 + 1) * S],
                in_=po[0:64, :],
            )

        def emit_norm(b, h):
            # respread row-sums onto 128 lanes via DRAM (reciprocal is ~8
            # cyc/elem/lane), broadcast the bf16 reciprocals back over 64
            # partitions, multiply in place (all-bf16 SBUF: 2x/4x DVE mode).
            hh = h % 2
            hp = h // 2
            sums_dr = sums_drs.pop((b, h))
            sums_sq = rpool.tile([128, S // 128], F32, tag="ssq", name=f"ssq{b}_{h}")
            nc.sync.dma_start(
                out=sums_sq, in_=sums_dr.rearrange("(p c) -> p c", p=128)
            )
            recs_sq = rpool.tile([128, S // 128], ADT, tag="rsq", name=f"rsq{b}_{h}")
            nc.vector.reciprocal(out=recs_sq, in_=sums_sq)
            recs_dr = drpool.tile([S], ADT, tag="rdr", name=f"rdr{b}_{h}")
            nc.sync.dma_start(
                out=recs_dr.rearrange("(p c) -> p c", p=128), in_=recs_sq
            )
            bc = bcpool.tile([128, S], ADT, tag="bc", name=f"bc{b}_{h}")
            nc.sync.dma_start(
                out=bc[hh * 64 : (hh + 1) * 64, :],
                in_=recs_dr[:].partition_broadcast(64),
            )
            nc.vector.tensor_mul(
                out=o_sbs[b][hh * 64 : (hh + 1) * 64, hp * S : (hp + 1) * S],
                in0=o_sbs[b][hh * 64 : (hh + 1) * 64, hp * S : (hp + 1) * S],
                in1=bc[hh * 64 : (hh + 1) * 64, :],
            )

        def emit_prologue(b, h):
            emit_s(b, h, 0)
            emit_s(b, h, 1)

        def emit_body(b, h, v_interleave=False, extras=None):
            for jc in range(SCJ):
                if v_interleave and jc < SCJ - 2:
                    emit_v_chunk(b, jc + 2)
                emit_p(b, h, jc)
                if jc + 2 < SCJ:
                    emit_s(b, h, jc + 2)
                if extras is not None and jc in extras:
                    extras[jc]()
            emit_evac(b, h)
            emit_norm(b, h)

        yq = [0]

        def emit_ob_chunk(b, sc):
            # full out-projection chunk for a finished batch
            o_sb = o_sbs[b]
            ps = ps_a.tile([128, 1024], F32, tag="ps_a", name=f"psy{b}_{sc}")
            for cc in range(CCH):
                nc.tensor.matmul(
                    ps[:, 0:512],
                    lhsT=o_sb[:, cc * S + sc * 128 : cc * S + (sc + 1) * 128],
                    rhs=wo_sb[:, cc * C : (cc + 1) * C],
                    start=(cc == 0),
                    stop=(cc == CCH - 1),
                )
            y_sb = ypool.tile([128, C], F32, tag="y", name=f"y{b}_{sc}")
            nc.vector.tensor_add(out=y_sb, in0=ps[:, 0:512], in1=beff_sb)
            eng = nc.gpsimd if (yq[0] % 2 == 0) else nc.sync
            yq[0] += 1
            eng.dma_start(out=y[b][sc * 128 : (sc + 1) * 128, :], in_=y_sb)

        ys = [None] * SCJ

        def emit_pyA(sc):
            # final batch out-proj, phase A: cc=0..2 staged into SBUF
            o_sb = o_sbs[BPC - 1]
            ps = ps_a.tile([128, 512], F32, tag="ps_a", name=f"pyA{sc}")
            for cc in range(CCH - 1):
                nc.tensor.matmul(
                    ps[:, 0:512],
                    lhsT=o_sb[:, cc * S + sc * 128 : cc * S + (sc + 1) * 128],
                    rhs=wo_sb[:, cc * C : (cc + 1) * C],
                    start=(cc == 0),
                    stop=(cc == CCH - 2),
                )
            y_sb = ypool.tile([128, C], F32, tag="y", name=f"yA{sc}")
            nc.vector.tensor_add(out=y_sb, in0=ps[:, 0:512], in1=beff_sb)
            ys[sc] = y_sb

        def emit_pyB(sc):
            # final batch out-proj, phase B: cc=3 added into the staged tiles
            o_sb = o_sbs[BPC - 1]
            cc = CCH - 1
            ps = ps_a.tile([128, 512], F32, tag="ps_a", name=f"pyB{sc}")
            nc.tensor.matmul(
                ps[:, 0:512],
                lhsT=o_sb[:, cc * S + sc * 128 : cc * S + (sc + 1) * 128],
                rhs=wo_sb[:, cc * C : (cc + 1) * C],
                start=True,
                stop=True,
            )
            nc.vector.tensor_add(out=ys[sc], in0=ys[sc], in1=ps[:, 0:512])
            eng = nc.gpsimd if (sc % 2 == 0) else nc.sync
            nc_eng = eng
            nc_eng.dma_start(out=y[BPC - 1][sc * 128 : (sc + 1) * 128, :], in_=ys[sc])

        # ---- main schedule -------------------------------------------------
        # filler units per (b, h), emitted before the head's body; sized to
        # the ACT slack (~2us/head) so the exp pipeline stays the pacer.
        fillers = {
            (0, 0): [],  # v(0,0), v(0,1) go here explicitly below
            (0, 1): [lambda: emit_qk_chunk(0, 1), lambda: emit_qk_chunk(0, 5)],
            (0, 2): [lambda: emit_qk_chunk(0, 2), lambda: emit_qk_chunk(0, 6)],
            (0, 3): [lambda: emit_qk_chunk(0, 3), lambda: emit_qk_chunk(0, 7)],
            (0, 4): [lambda: emit_qk_chunk(1, 0)],
            (0, 5): [lambda: emit_qk_chunk(1, 4)],
            (0, 6): [],
            (0, 7): [],
            (1, 0): [],
            (1, 1): [lambda: emit_qk_chunk(1, 1), lambda: emit_qk_chunk(1, 5)],
            (1, 2): [lambda: emit_qk_chunk(1, 2), lambda: emit_qk_chunk(1, 6)],
            (1, 3): [lambda: emit_qk_chunk(1, 3), lambda: emit_qk_chunk(1, 7)],
            (1, 4): [lambda: emit_ob_chunk(0, 0), lambda: emit_ob_chunk(0, 1)],
            (1, 5): [lambda: emit_ob_chunk(0, 2), lambda: emit_ob_chunk(0, 3)],
            (1, 6): [lambda: emit_ob_chunk(0, 4), lambda: emit_ob_chunk(0, 5)],
            (1, 7): [lambda: emit_ob_chunk(0, 6), lambda: emit_ob_chunk(0, 7)],
        }

        # boot: first q/k chunks, then head 0's pipeline prologue
        emit_qk_chunk(0, 0)
        emit_qk_chunk(0, 4)
        for b in range(BPC):
            o_sbs[b] = opool.tile([128, CCH * S], ADT, tag="o", name=f"o{b}")
            last_b = b == BPC - 1
            if b == 0:
                emit_v_alloc(0)
                emit_prologue(0, 0)
            for h in range(H):
                for u in fillers[(b, h)]:
                    u()
                if h == 0:
                    emit_v_chunk(b, 0)
                    emit_v_chunk(b, 1)
                extras = None
                if last_b and h == H - 1:
                    # stage phase-A out-proj chunks inside head 7's slack
                    extras = {jc: (lambda sc=jc - 2: emit_pyA(sc)) for jc in range(2, 8)}
                emit_body(b, h, v_interleave=(h == 0), extras=extras)
                if b == 0 and h == 2:
                    emit_x(1)  # next batch's x DMA, early
                if h < H - 1:
                    emit_prologue(b, h + 1)
                elif not last_b:
                    emit_v_alloc(b + 1)
                    emit_prologue(b + 1, 0)
        # tail: remaining phase-A chunks, then cc=3 contributions + stores
        emit_pyA(6)
        emit_pyA(7)
        for sc in range(SCJ):
            emit_pyB(sc)

    nc.compile()
    return nc


_NC_CACHE = None
LAST_RESULT = None


def kernel(vis_feat, text_feat, w_qkv, b_qkv, w_out, b_out):
    global _NC_CACHE, LAST_RESULT
    _register_ntff_hook()
    if _NC_CACHE is None:
        _NC_CACHE = build()
    nc = _NC_CACHE

    adt_np = np.dtype(mybir.dt.np(ADT))
    vis_feat = np.asarray(vis_feat, dtype=np.float32)
    w_qkv = np.asarray(w_qkv, dtype=np.float32)
    b_qkv = np.asarray(b_qkv, dtype=np.float32)
    w_out = np.asarray(w_out, dtype=np.float32)
    b_out = np.asarray(b_out, dtype=np.float32)

    wqkvT = np.ascontiguousarray(w_qkv.T).astype(adt_np)  # [C, 3C]
    wouT = np.ascontiguousarray(w_out.T).astype(adt_np)  # [C, C]
    bqk = np.ascontiguousarray(b_qkv[: 2 * C].reshape(FCH, 128).T)  # [128, 8]
    beff = np.ascontiguousarray(b_out + b_qkv[2 * C :] @ w_out.T)  # [C]

    in_maps = []
    for i in range(NCORES):
        xTi = np.ascontiguousarray(
            vis_feat[i * BPC : (i + 1) * BPC].transpose(0, 2, 1)
        ).astype(adt_np)  # [BPC, C, S]
        in_maps.append(
            {"xT": xTi, "wqkvT": wqkvT, "wouT": wouT, "bqk": bqk, "beff": beff}
        )

    res = run_bass_kernel_spmd(nc, in_maps, core_ids=list(range(NCORES)))
    LAST_RESULT = res
    return np.concatenate([res.results[i]["y"] for i in range(NCORES)], axis=0)
